# revision 1
# baseline (speedup 1.0000x reference)
"""Trainium2 Bass kernel for nn_BiLingual (dual embedding gather + cAddTanh pool).

Computes, for two embedding tables:
    out[t, b, :] = sum_{j=0}^{S-2} tanh(W_t[idx_t[b, j]] + W_t[idx_t[b, j+1]])

Sharding: data-parallel over batch. Each of the 8 cores handles 8 batch rows
for BOTH tables; tables are replicated.

Per-core device pipeline:
  1. dma_gather (gpsimd custom ucode): embedding rows land position-on-
     partition in overlap-by-1 groups of 128 positions (group g covers
     positions [127g, 127g+128)), 17 groups per sequence row.  The int16
     index range is handled by biasing: base = W[32768:] and signed
     idx' = idx - 32768 in [-32768, 17231] -- the ucode does plain signed
     address arithmetic (HW-verified), so one pass covers all 50000 rows.
     HW constraints handled:
       - <= 1024 indices per dma_gather (SWDGE ring), so each row's 17
         groups split into 3 gathers (7+7+3 groups).
       - trailing-negative indices are trimmed as padding, so streams 1-2
         append 16 zero-guard indices (their junk slot is overwritten by
         the next gather) and stream 3 ends with group 16 whose tail
         positions (>= S) are set to index 0 and masked out of the reduce.
  2. PE shift-add matmul with lhsT[k,m] = (k==m) + (k==m+1):
     A[p,:] = E[p,:] + E[p+1,:]  (pair j = 127g + p, valid p<127).
  3. ACT tanh PSUM -> SBUF.
  4. PE masked ones-matmul reduces tanh values over valid positions into a
     [16, 256] PSUM accumulator (output partition = table*8 + local_row).
"""
import os

import numpy as np

from concourse import bacc, mybir
import concourse.tile as tile
from concourse.bass_utils import run_bass_kernel_spmd

P = 128
B, S, V, D = 64, 2048, 50000, 256
N_CORES = 8
B_LOC = B // N_CORES        # 8 batch rows per core
G = 17                      # overlap-by-1 groups of 128 per sequence row
NROW = 2 * B_LOC            # 16 (table, local row) pairs per core
SPLIT = 32768
CHUNK_GROUPS = 4            # groups per PSUM/tanh chunk

# per-row gather streams: (first slot-group, n groups, n stream idxs incl guard)
STREAMS = [(0, 7, 7 * P + 16), (7, 7, 7 * P + 16), (14, 3, 3 * P)]
IDX_COLS = 64               # idx tile column pitch per stream (aligned)
N_SLOTS = NROW * len(STREAMS)

_last_results = None        # set by _run for test harness introspection


def _build_positions():
    # POS[p, g] = min(127*g + p, S-1)
    p = np.arange(P)[:, None]
    g = np.arange(G)[None, :]
    return np.minimum(127 * g + p, S - 1)


_POS = _build_positions()


def _build_shiftT():
    # lhsT for A = M2 @ E with M2[m,m]=1, M2[m,m+1]=1  =>  lhsT[k,m] = M2[m,k]
    m = np.zeros((P, P), dtype=np.float32)
    k = np.arange(P)
    m[k, k] = 1.0
    m[k[1:], k[1:] - 1] = 1.0
    return m


def _build_red_masks():
    # red[:, (row16*2 + ty)*16 : +16]: column row16 holds mask_ty, rest 0.
    # ty=0: valid pairs p < 127 (full group); ty=1: p < 15 (tail group 16).
    red = np.zeros((P, NROW * 2 * 16), dtype=np.float32)
    masks = [
        (np.arange(P) < 127).astype(np.float32),
        (np.arange(P) < 15).astype(np.float32),
    ]
    for row16 in range(NROW):
        for ty in range(2):
            red[:, (row16 * 2 + ty) * 16 + row16] = masks[ty]
    return red


def _split_multi_waits(nc, max_waits=1):
    """Walrus rejects instructions carrying too many sync waits; hoist excess
    waits onto same-engine NOPs inserted just before the instruction (engine
    program order makes this equivalent)."""
    for bb in nc.main_func.blocks:
        idx = 0
        while idx < len(bb.instructions):
            ins = bb.instructions[idx]
            si = ins.sync_info
            if si is not None and si.on_wait and len(si.on_wait) > max_waits:
                waits = list(si.on_wait)
                extra, keep = waits[:-max_waits], waits[-max_waits:]
                for w0 in range(0, len(extra), max_waits):
                    nop = mybir.InstNoOp(
                        name=nc.get_next_instruction_name(), ins=[], outs=[]
                    )
                    nop.engine = ins.engine
                    nop.sync_info = mybir.SyncInfo(
                        on_wait=extra[w0 : w0 + max_waits], on_update=[]
                    )
                    nc.register_instruction(nop)
                    bb.instructions.insert(idx, nop)
                    idx += 1
                si.on_wait = keep
            idx += 1


def _build_program():
    stage = os.environ.get("KBISECT", "full")  # gather | tanh | full
    nc = bacc.Bacc(None, target_bir_lowering=False)
    Wp = nc.declare_dram_parameter("W_pri", [V, D], mybir.dt.float32, isOutput=False)
    Ws = nc.declare_dram_parameter("W_sec", [V, D], mybir.dt.float32, isOutput=False)
    idxA = nc.declare_dram_parameter(
        "idxA", [P, N_SLOTS * IDX_COLS], mybir.dt.int16, isOutput=False
    )
    shiftT = nc.declare_dram_parameter("shiftT", [P, P], mybir.dt.float32, isOutput=False)
    red = nc.declare_dram_parameter(
        "red", [P, NROW * 2 * 16], mybir.dt.float32, isOutput=False
    )
    out = nc.declare_dram_parameter("out", [NROW, D], mybir.dt.float32, isOutput=True)

    with tile.TileContext(nc) as tc:
        with (
            tc.tile_pool(name="const", bufs=1) as const,
            tc.tile_pool(name="ebuf", bufs=3) as ebuf,
            tc.tile_pool(name="tbuf", bufs=3) as tbuf,
            tc.tile_pool(name="psA", bufs=3, space="PSUM") as psA,
            tc.tile_pool(name="psR", bufs=1, space="PSUM") as psR,
            tc.tile_pool(name="osb", bufs=1) as osb,
        ):
            shift_t = const.tile([P, P], mybir.dt.float32)
            nc.sync.dma_start(out=shift_t[:], in_=shiftT[:])
            red_t = const.tile([P, NROW * 2 * 16], mybir.dt.float32)
            nc.sync.dma_start(out=red_t[:], in_=red[:])
            iA = const.tile([P, N_SLOTS * IDX_COLS], mybir.dt.int16)
            nc.sync.dma_start(out=iA[:], in_=idxA[:])

            acc = psR.tile([NROW, D], mybir.dt.float32, space="PSUM")
            n_red = NROW * G
            red_i = 0
            last_e = last_tt = None

            for t, W in enumerate((Wp, Ws)):
                for r in range(B_LOC):
                    row16 = t * B_LOC + r
                    e = ebuf.tile([P, G, D], mybir.dt.float32)
                    for k, (g0, ngrp, nidx) in enumerate(STREAMS):
                        slot = row16 * len(STREAMS) + k
                        dst_hi = g0 + (nidx + P - 1) // P
                        nc.gpsimd.dma_gather(
                            out_ap=e[:, g0:dst_hi, :],
                            in_ap=W[SPLIT:, :],
                            idxs_ap=iA[
                                :, slot * IDX_COLS : slot * IDX_COLS + nidx // 16
                            ],
                            num_idxs=nidx,
                            num_idxs_reg=nidx,
                            elem_size=D,
                        )
                    ef = e[:].rearrange("p g d -> p (g d)")
                    last_e = e
                    if stage == "gather":
                        continue
                    for c0 in range(0, G, CHUNK_GROUPS):
                        ng = min(CHUNK_GROUPS, G - c0)
                        a = psA.tile(
                            [P, CHUNK_GROUPS * D], mybir.dt.float32, space="PSUM"
                        )
                        for h0 in range(0, ng, 2):
                            nh = min(2, ng - h0)
                            nc.tensor.matmul(
                                out=a[:, h0 * D : (h0 + nh) * D],
                                lhsT=shift_t[:],
                                rhs=ef[:, (c0 + h0) * D : (c0 + h0 + nh) * D],
                                start=True,
                                stop=True,
                            )
                        tt = tbuf.tile([P, CHUNK_GROUPS * D], mybir.dt.float32)
                        nc.scalar.activation(
                            tt[:, : ng * D],
                            a[:, : ng * D],
                            mybir.ActivationFunctionType.Tanh,
                        )
                        last_tt = tt
                        if stage == "tanh":
                            continue
                        for gi in range(ng):
                            gg = c0 + gi
                            ty = 1 if gg == G - 1 else 0
                            nc.tensor.matmul(
                                out=acc[:],
                                lhsT=red_t[
                                    :, (row16 * 2 + ty) * 16 : (row16 * 2 + ty + 1) * 16
                                ],
                                rhs=tt[:, gi * D : (gi + 1) * D],
                                start=(red_i == 0),
                                stop=(red_i == n_red - 1),
                            )
                            red_i += 1

            res_sb = osb.tile([NROW, D], mybir.dt.float32)
            if stage == "gather":
                nc.scalar.copy(out=res_sb[:], in_=last_e[0:NROW, 0, :])
            elif stage == "tanh":
                nc.scalar.copy(out=res_sb[:], in_=last_tt[0:NROW, 0:D])
            else:
                nc.scalar.copy(out=res_sb[:], in_=acc[:])
            nc.sync.dma_start(out=out[:], in_=res_sb[:])

    nc.compile()
    _split_multi_waits(nc)
    return nc


def _host_prep(inputs_pri, inputs_sec, W_pri, W_sec):
    ip = np.asarray(inputs_pri).astype(np.int64, copy=False)
    is_ = np.asarray(inputs_sec).astype(np.int64, copy=False)
    wp = np.ascontiguousarray(np.asarray(W_pri, dtype=np.float32))
    ws = np.ascontiguousarray(np.asarray(W_sec, dtype=np.float32))
    shiftT = _build_shiftT()
    red = _build_red_masks()

    in_maps = []
    for k in range(N_CORES):
        idxA = np.zeros((P, N_SLOTS * IDX_COLS), dtype=np.int16)
        for t, idx in enumerate((ip, is_)):
            for r in range(B_LOC):
                row16 = t * B_LOC + r
                vgp = (idx[k * B_LOC + r][_POS].T - SPLIT).astype(np.int16)  # [G, P]
                vgp[G - 1, 16:] = 0  # controllable tail of group 16
                for s, (g0, ngrp, nidx) in enumerate(STREAMS):
                    stream = vgp[g0 : g0 + ngrp].reshape(-1)
                    if nidx > ngrp * P:
                        stream = np.concatenate(
                            [stream, np.zeros(nidx - ngrp * P, np.int16)]
                        )
                    slot = row16 * len(STREAMS) + s
                    wrapped = np.tile(stream.reshape(-1, 16).T, (8, 1))
                    idxA[:, slot * IDX_COLS : slot * IDX_COLS + nidx // 16] = wrapped
        in_maps.append(
            {
                "W_pri": wp,
                "W_sec": ws,
                "idxA": idxA,
                "shiftT": shiftT,
                "red": red,
            }
        )
    return in_maps


def _run(inputs_pri, inputs_sec, W_pri, W_sec, trace=False):
    global _last_results
    nc = _build_program()
    in_maps = _host_prep(inputs_pri, inputs_sec, W_pri, W_sec)
    res = run_bass_kernel_spmd(nc, in_maps, list(range(N_CORES)), trace=trace)
    _last_results = res
    out = np.empty((2, B, D), dtype=np.float32)
    for k in range(N_CORES):
        o = res.results[k]["out"]  # [16, 256]
        out[0, k * B_LOC : (k + 1) * B_LOC] = o[:B_LOC]
        out[1, k * B_LOC : (k + 1) * B_LOC] = o[B_LOC:]
    return out


def kernel(inputs_pri, inputs_sec, W_pri, W_sec):
    trace = bool(int(os.environ.get("KERNEL_TRACE", "0")))
    return _run(inputs_pri, inputs_sec, W_pri, W_sec, trace=trace)



# revision 6
# speedup vs baseline: 4.1125x; 4.1125x over previous
"""Trainium2 Bass kernel for nn_BiLingual (dual embedding gather + cAddTanh pool).

Computes, for two embedding tables:
    out[t, b, :] = sum_{j=0}^{S-2} tanh(W_t[idx_t[b, j]] + W_t[idx_t[b, j+1]])

Sharding: data-parallel over batch. Each of the 8 cores handles 8 batch rows
for BOTH tables; tables are replicated (converted to bf16 on the host).

Per-core device layout (v3): partition p owns positions [16p, 16p+16) of the
sequence; gathered rows land column-major (slot s -> partition s%128, column
s//128, position = 16*(s%128) + s//128 + 16*c0).  Consecutive positions of a
pair then sit on the SAME partition in ADJACENT columns, so the pairwise add
is a legal free-axis-offset DVE op.  The per-partition boundary pair
(16p+15, 16p+16) gets its right element from one extra 128-idx gather.

Pipeline per sequence row:
  1. dma_gather (gpsimd SWDGE, bf16 tables, 512B/descriptor):
       stream A: cols c=0..6   (896 idxs + 16 biased-0 guards -> junk col 7)
       stream B: cols c=7..13  (896 idxs + 16 guards -> junk col 15)
       stream C: cols c=14,15 + boundary rights pos 16p+16 (384 idxs ->
                 cols 16-18; final slot is masked and forced to biased-0 so
                 the ucode's trailing-negative trim never fires).
     num_idxs > 1024 hangs the SWDGE ucode (HW-bisected) -> streams <= 912.
     Streams round-robin over the 4 SWDGE queues so all four Q7 core pairs
     generate descriptors concurrently.  int16 index range via biasing:
     base = W[32768:], idx' = idx - 32768.
  2. DVE adds (free-axis column offsets, all partitions base-0):
       A[:, 0:6]   = e[:, 0:6]   + e[:, 1:7]    (pairs c=0..5)
       A[:, 6]     = e[:, 6]     + e[:, 8]      (pair  c=6, skips junk col)
       A[:, 7:13]  = e[:, 8:14]  + e[:, 9:15]   (pairs c=7..12)
       A[:, 13]    = e[:, 14]    + e[:, 16]     (pair  c=13, skips junk col)
       A[:, 14:16] = e[:, 16:18] + e[:, 17:19]  (c=14 + boundary, valid p<127
                                                 for the boundary column)
  3. ACT tanh A -> T (bf16), one [128, 4096] instruction per sequence.
  4. PE masked ones-matmul reduces T into a [16, 256] PSUM accumulator
     (output partition = table*8 + local_row); mask ty0 = all partitions
     (in-partition pairs), ty1 = p<127 (boundary column).
"""
import os

import numpy as np

from concourse import bacc, mybir
import concourse.tile as tile
from concourse.bass_utils import run_bass_kernel_spmd

P = 128
B, S, V, D = 64, 2048, 50000, 256
N_CORES = 8
B_LOC = B // N_CORES        # 8 batch rows per core
CPP = 16                    # positions per partition
NCOL = 16                   # result pair-columns per sequence (15 main + 1 boundary)
NROW = 2 * B_LOC            # 16 (table, local row) pairs per core
SPLIT = 32768
N_QUEUES = int(os.environ.get("KQUEUES", "4"))

# per-row gather streams: (first pos-col, n idxs incl guards, dst col, dst ncol)
# num_idxs > 1024 hangs the SWDGE ucode (HW-bisected), so streams stay <= 912/384.
STREAMS = [(0, 7 * P + 16, 0, 8), (7, 7 * P + 16, 8, 8), (-1, 3 * P, 16, 3)]
IDX_COLS = 64               # idx tile column pitch per stream (>= 912/16, 32B-aligned)
N_SLOTS = NROW * len(STREAMS)

_last_results = None        # set by _run for test harness introspection


def _build_red_masks():
    # red[:, (row16*2 + ty)*16 : +16]: column row16 holds mask_ty, rest 0.
    # ty=0: all partitions valid (in-partition pairs); ty=1: p < 127 (boundary).
    red = np.zeros((P, NROW * 2 * 16), dtype=np.float32)
    masks = [
        np.ones(P, dtype=np.float32),
        (np.arange(P) < 127).astype(np.float32),
    ]
    for row16 in range(NROW):
        for ty in range(2):
            red[:, (row16 * 2 + ty) * 16 + row16] = masks[ty]
    return red


def _split_multi_waits(nc, max_waits=1):
    """Walrus rejects instructions carrying too many sync waits; hoist excess
    waits onto same-engine NOPs inserted just before the instruction (engine
    program order makes this equivalent)."""
    for bb in nc.main_func.blocks:
        idx = 0
        while idx < len(bb.instructions):
            ins = bb.instructions[idx]
            si = ins.sync_info
            if si is not None and si.on_wait and len(si.on_wait) > max_waits:
                waits = list(si.on_wait)
                extra, keep = waits[:-max_waits], waits[-max_waits:]
                for w0 in range(0, len(extra), max_waits):
                    nop = mybir.InstNoOp(
                        name=nc.get_next_instruction_name(), ins=[], outs=[]
                    )
                    nop.engine = ins.engine
                    nop.sync_info = mybir.SyncInfo(
                        on_wait=extra[w0 : w0 + max_waits], on_update=[]
                    )
                    nc.register_instruction(nop)
                    bb.instructions.insert(idx, nop)
                    idx += 1
                si.on_wait = keep
            idx += 1


def _build_program():
    nc = bacc.Bacc(None, target_bir_lowering=False, num_swdge_queues=N_QUEUES)
    bf16 = mybir.dt.bfloat16
    Wp = nc.declare_dram_parameter("W_pri", [V, D], bf16, isOutput=False)
    Ws = nc.declare_dram_parameter("W_sec", [V, D], bf16, isOutput=False)
    idxA = nc.declare_dram_parameter(
        "idxA", [P, N_SLOTS * IDX_COLS], mybir.dt.int16, isOutput=False
    )
    red = nc.declare_dram_parameter(
        "red", [P, NROW * 2 * 16], mybir.dt.float32, isOutput=False
    )
    out = nc.declare_dram_parameter("out", [NROW, D], mybir.dt.float32, isOutput=True)

    with tile.TileContext(nc) as tc:
        with (
            tc.tile_pool(name="const", bufs=1) as const,
            tc.tile_pool(name="ebuf", bufs=3) as ebuf,
            tc.tile_pool(name="abuf", bufs=3) as abuf,
            tc.tile_pool(name="tbuf", bufs=3) as tbuf,
            tc.tile_pool(name="psR", bufs=1, space="PSUM") as psR,
            tc.tile_pool(name="osb", bufs=1) as osb,
        ):
            red_f32 = const.tile([P, NROW * 2 * 16], mybir.dt.float32)
            nc.sync.dma_start(out=red_f32[:], in_=red[:])
            red_t = const.tile([P, NROW * 2 * 16], bf16)
            nc.vector.tensor_copy(out=red_t[:], in_=red_f32[:])
            iA = const.tile([P, N_SLOTS * IDX_COLS], mybir.dt.int16)
            nc.sync.dma_start(out=iA[:], in_=idxA[:])

            acc = psR.tile([NROW, D], mybir.dt.float32, space="PSUM")
            n_red = NROW * NCOL
            red_i = 0
            q = 0

            for t, W in enumerate((Wp, Ws)):
                for r in range(B_LOC):
                    row16 = t * B_LOC + r
                    e = ebuf.tile([P, 19, D], bf16)
                    for k, (c0, nidx, d0, ncol) in enumerate(STREAMS):
                        slot = row16 * len(STREAMS) + k
                        nc.gpsimd.dma_gather(
                            out_ap=e[:, d0 : d0 + ncol, :],
                            in_ap=W[SPLIT:, :],
                            idxs_ap=iA[
                                :, slot * IDX_COLS : slot * IDX_COLS + nidx // 16
                            ],
                            num_idxs=nidx,
                            num_idxs_reg=nidx,
                            elem_size=D,
                            queue_num=q % N_QUEUES,
                        )
                        q += 1
                    a = abuf.tile([P, NCOL, D], bf16)
                    nc.vector.tensor_add(
                        out=a[:, 0:6, :], in0=e[:, 0:6, :], in1=e[:, 1:7, :]
                    )
                    nc.vector.tensor_add(
                        out=a[:, 6:7, :], in0=e[:, 6:7, :], in1=e[:, 8:9, :]
                    )
                    nc.vector.tensor_add(
                        out=a[:, 7:13, :], in0=e[:, 8:14, :], in1=e[:, 9:15, :]
                    )
                    nc.vector.tensor_add(
                        out=a[:, 13:14, :], in0=e[:, 14:15, :], in1=e[:, 16:17, :]
                    )
                    nc.vector.tensor_add(
                        out=a[:, 14:16, :], in0=e[:, 16:18, :], in1=e[:, 17:19, :]
                    )
                    tt = tbuf.tile([P, NCOL, D], bf16)
                    nc.scalar.activation(
                        tt[:].rearrange("p g d -> p (g d)"),
                        a[:].rearrange("p g d -> p (g d)"),
                        mybir.ActivationFunctionType.Tanh,
                    )
                    for g in range(NCOL):
                        ty = 1 if g == NCOL - 1 else 0
                        nc.tensor.matmul(
                            out=acc[:],
                            lhsT=red_t[
                                :, (row16 * 2 + ty) * 16 : (row16 * 2 + ty + 1) * 16
                            ],
                            rhs=tt[:, g, :],
                            start=(red_i == 0),
                            stop=(red_i == n_red - 1),
                        )
                        red_i += 1

            res_sb = osb.tile([NROW, D], mybir.dt.float32)
            nc.scalar.copy(out=res_sb[:], in_=acc[:])
            nc.sync.dma_start(out=out[:], in_=res_sb[:])

    nc.compile()
    _split_multi_waits(nc)
    return nc


def _host_prep(inputs_pri, inputs_sec, W_pri, W_sec):
    import ml_dtypes

    ip = np.asarray(inputs_pri).astype(np.int64, copy=False)
    is_ = np.asarray(inputs_sec).astype(np.int64, copy=False)
    wp = np.ascontiguousarray(np.asarray(W_pri, dtype=np.float32)).astype(
        ml_dtypes.bfloat16
    )
    ws = np.ascontiguousarray(np.asarray(W_sec, dtype=np.float32)).astype(
        ml_dtypes.bfloat16
    )
    red = _build_red_masks()

    p_ar = np.arange(P)
    in_maps = []
    for k in range(N_CORES):
        idxA = np.zeros((P, N_SLOTS * IDX_COLS), dtype=np.int16)
        for t, idx in enumerate((ip, is_)):
            for r in range(B_LOC):
                row16 = t * B_LOC + r
                seq = idx[k * B_LOC + r]  # [S]
                for s, (c0, nidx, d0, ncol) in enumerate(STREAMS):
                    if c0 >= 0:
                        # slot s -> partition s%128, col c0 + s//128,
                        # position 16*(s%128) + (c0 + s//128)
                        pos = (CPP * p_ar[None, :] + c0 + np.arange(7)[:, None]).reshape(
                            -1
                        )  # [896] in slot order (col-major)
                        stream = (seq[pos] - SPLIT).astype(np.int16)
                        stream = np.concatenate(
                            [stream, np.zeros(nidx - 7 * P, np.int16)]
                        )
                    else:
                        # cols c14, c15, then boundary rights (pos 16p+16,
                        # clamped); final slot (p=127) is masked out of the
                        # reduce -> biased-0 so the trailing-negative trim
                        # never fires.
                        pos = np.concatenate(
                            [
                                CPP * p_ar + 14,
                                CPP * p_ar + 15,
                                np.minimum(CPP * p_ar + CPP, S - 1),
                            ]
                        )
                        stream = (seq[pos] - SPLIT).astype(np.int16)
                        stream[3 * P - 1] = 0
                    slot = row16 * len(STREAMS) + s
                    wrapped = np.tile(stream.reshape(-1, 16).T, (8, 1))
                    idxA[:, slot * IDX_COLS : slot * IDX_COLS + nidx // 16] = wrapped
        in_maps.append(
            {
                "W_pri": wp,
                "W_sec": ws,
                "idxA": idxA,
                "red": red,
            }
        )
    return in_maps


def _run(inputs_pri, inputs_sec, W_pri, W_sec, trace=False):
    global _last_results
    nc = _build_program()
    in_maps = _host_prep(inputs_pri, inputs_sec, W_pri, W_sec)
    res = run_bass_kernel_spmd(nc, in_maps, list(range(N_CORES)), trace=trace)
    _last_results = res
    out = np.empty((2, B, D), dtype=np.float32)
    for k in range(N_CORES):
        o = res.results[k]["out"]  # [16, 256]
        out[0, k * B_LOC : (k + 1) * B_LOC] = o[:B_LOC]
        out[1, k * B_LOC : (k + 1) * B_LOC] = o[B_LOC:]
    return out


def kernel(inputs_pri, inputs_sec, W_pri, W_sec):
    trace = bool(int(os.environ.get("KERNEL_TRACE", "0")))
    return _run(inputs_pri, inputs_sec, W_pri, W_sec, trace=trace)


# revision 7
# speedup vs baseline: 4.2174x; 1.0255x over previous
"""Trainium2 Bass kernel for nn_BiLingual (dual embedding gather + cAddTanh pool).

Computes, for two embedding tables:
    out[t, b, :] = sum_{j=0}^{S-2} tanh(W_t[idx_t[b, j]] + W_t[idx_t[b, j+1]])

Sharding: data-parallel over batch. Each of the 8 cores handles 8 batch rows
for BOTH tables; tables are replicated (converted to bf16 on the host).

Per-core device layout (v3): partition p owns positions [16p, 16p+16) of the
sequence; gathered rows land column-major (slot s -> partition s%128, column
s//128, position = 16*(s%128) + s//128 + 16*c0).  Consecutive positions of a
pair then sit on the SAME partition in ADJACENT columns, so the pairwise add
is a legal free-axis-offset DVE op.  The per-partition boundary pair
(16p+15, 16p+16) gets its right element from one extra 128-idx gather.

Pipeline per sequence row:
  1. dma_gather (gpsimd SWDGE, bf16 tables, 512B/descriptor):
       stream A: cols c=0..6   (896 idxs + 16 biased-0 guards -> junk col 7)
       stream B: cols c=7..13  (896 idxs + 16 guards -> junk col 15)
       stream C: cols c=14,15 + boundary rights pos 16p+16 (384 idxs ->
                 cols 16-18; final slot is masked and forced to biased-0 so
                 the ucode's trailing-negative trim never fires).
     num_idxs > 1024 hangs the SWDGE ucode (HW-bisected) -> streams <= 912.
     Streams round-robin over the 4 SWDGE queues so all four Q7 core pairs
     generate descriptors concurrently.  int16 index range via biasing:
     base = W[32768:], idx' = idx - 32768.
  2. DVE adds (free-axis column offsets, all partitions base-0):
       A[:, 0:6]   = e[:, 0:6]   + e[:, 1:7]    (pairs c=0..5)
       A[:, 6]     = e[:, 6]     + e[:, 8]      (pair  c=6, skips junk col)
       A[:, 7:13]  = e[:, 8:14]  + e[:, 9:15]   (pairs c=7..12)
       A[:, 13]    = e[:, 14]    + e[:, 16]     (pair  c=13, skips junk col)
       A[:, 14:16] = e[:, 16:18] + e[:, 17:19]  (c=14 + boundary, valid p<127
                                                 for the boundary column)
  3. ACT tanh A -> T (bf16), one [128, 4096] instruction per sequence.
  4. PE masked ones-matmul reduces T into a [16, 256] PSUM accumulator
     (output partition = table*8 + local_row); mask ty0 = all partitions
     (in-partition pairs), ty1 = p<127 (boundary column).
"""
import os

import numpy as np

from concourse import bacc, mybir
import concourse.tile as tile
from concourse.bass_utils import run_bass_kernel_spmd

P = 128
B, S, V, D = 64, 2048, 50000, 256
N_CORES = 8
B_LOC = B // N_CORES        # 8 batch rows per core
CPP = 16                    # positions per partition
NCOL = 16                   # result pair-columns per sequence (15 main + 1 boundary)
NROW = 2 * B_LOC            # 16 (table, local row) pairs per core
SPLIT = 32768
N_QUEUES = int(os.environ.get("KQUEUES", "4"))

# per-row gather streams: (first pos-col, n idxs incl guards, dst col, dst ncol)
# num_idxs > 1024 hangs the SWDGE ucode (HW-bisected), so streams stay <= 912/384.
STREAMS = [(0, 7 * P + 16, 0, 8), (7, 7 * P + 16, 8, 8), (-1, 3 * P, 16, 3)]
IDX_COLS = 64               # idx tile column pitch per stream (>= 912/16, 32B-aligned)
N_SLOTS = NROW * len(STREAMS)

_last_results = None        # set by _run for test harness introspection


def _build_red_masks():
    # red[:, (row16*2 + ty)*16 : +16]: column row16 holds mask_ty, rest 0.
    # ty=0: all partitions valid (in-partition pairs); ty=1: p < 127 (boundary).
    red = np.zeros((P, NROW * 2 * 16), dtype=np.float32)
    masks = [
        np.ones(P, dtype=np.float32),
        (np.arange(P) < 127).astype(np.float32),
    ]
    for row16 in range(NROW):
        for ty in range(2):
            red[:, (row16 * 2 + ty) * 16 + row16] = masks[ty]
    return red


def _split_multi_waits(nc, max_waits=1):
    """Walrus rejects instructions carrying too many sync waits; hoist excess
    waits onto same-engine NOPs inserted just before the instruction (engine
    program order makes this equivalent)."""
    for bb in nc.main_func.blocks:
        idx = 0
        while idx < len(bb.instructions):
            ins = bb.instructions[idx]
            si = ins.sync_info
            if si is not None and si.on_wait and len(si.on_wait) > max_waits:
                waits = list(si.on_wait)
                extra, keep = waits[:-max_waits], waits[-max_waits:]
                for w0 in range(0, len(extra), max_waits):
                    nop = mybir.InstNoOp(
                        name=nc.get_next_instruction_name(), ins=[], outs=[]
                    )
                    nop.engine = ins.engine
                    nop.sync_info = mybir.SyncInfo(
                        on_wait=extra[w0 : w0 + max_waits], on_update=[]
                    )
                    nc.register_instruction(nop)
                    bb.instructions.insert(idx, nop)
                    idx += 1
                si.on_wait = keep
            idx += 1


def _build_program():
    nc = bacc.Bacc(None, target_bir_lowering=False, num_swdge_queues=N_QUEUES)
    bf16 = mybir.dt.bfloat16
    Wp = nc.declare_dram_parameter("W_pri", [V, D], bf16, isOutput=False)
    Ws = nc.declare_dram_parameter("W_sec", [V, D], bf16, isOutput=False)
    idxA = nc.declare_dram_parameter(
        "idxA", [P, N_SLOTS * IDX_COLS], mybir.dt.int16, isOutput=False
    )
    red = nc.declare_dram_parameter(
        "red", [P, NROW * 2 * 16], mybir.dt.float32, isOutput=False
    )
    out = nc.declare_dram_parameter("out", [NROW, D], mybir.dt.float32, isOutput=True)

    with tile.TileContext(nc) as tc:
        with (
            tc.tile_pool(name="const", bufs=1) as const,
            tc.tile_pool(name="ebuf", bufs=6) as ebuf,
            tc.tile_pool(name="abuf", bufs=4) as abuf,
            tc.tile_pool(name="tbuf", bufs=4) as tbuf,
            tc.tile_pool(name="psR", bufs=1, space="PSUM") as psR,
            tc.tile_pool(name="osb", bufs=1) as osb,
        ):
            red_f32 = const.tile([P, NROW * 2 * 16], mybir.dt.float32)
            nc.sync.dma_start(out=red_f32[:], in_=red[:])
            red_t = const.tile([P, NROW * 2 * 16], bf16)
            nc.vector.tensor_copy(out=red_t[:], in_=red_f32[:])
            iA = const.tile([P, N_SLOTS * IDX_COLS], mybir.dt.int16)
            nc.sync.dma_start(out=iA[:], in_=idxA[:])

            acc = psR.tile([NROW, D], mybir.dt.float32, space="PSUM")
            n_red = NROW * NCOL
            red_i = 0
            q = 0

            for t, W in enumerate((Wp, Ws)):
                for r in range(B_LOC):
                    row16 = t * B_LOC + r
                    e = ebuf.tile([P, 19, D], bf16)
                    for k, (c0, nidx, d0, ncol) in enumerate(STREAMS):
                        slot = row16 * len(STREAMS) + k
                        nc.gpsimd.dma_gather(
                            out_ap=e[:, d0 : d0 + ncol, :],
                            in_ap=W[SPLIT:, :],
                            idxs_ap=iA[
                                :, slot * IDX_COLS : slot * IDX_COLS + nidx // 16
                            ],
                            num_idxs=nidx,
                            num_idxs_reg=nidx,
                            elem_size=D,
                            queue_num=q % N_QUEUES,
                        )
                        q += 1
                    a = abuf.tile([P, NCOL, D], bf16)
                    nc.vector.tensor_add(
                        out=a[:, 0:6, :], in0=e[:, 0:6, :], in1=e[:, 1:7, :]
                    )
                    nc.vector.tensor_add(
                        out=a[:, 6:7, :], in0=e[:, 6:7, :], in1=e[:, 8:9, :]
                    )
                    nc.vector.tensor_add(
                        out=a[:, 7:13, :], in0=e[:, 8:14, :], in1=e[:, 9:15, :]
                    )
                    nc.vector.tensor_add(
                        out=a[:, 13:14, :], in0=e[:, 14:15, :], in1=e[:, 16:17, :]
                    )
                    nc.vector.tensor_add(
                        out=a[:, 14:16, :], in0=e[:, 16:18, :], in1=e[:, 17:19, :]
                    )
                    tt = tbuf.tile([P, NCOL, D], bf16)
                    nc.scalar.activation(
                        tt[:].rearrange("p g d -> p (g d)"),
                        a[:].rearrange("p g d -> p (g d)"),
                        mybir.ActivationFunctionType.Tanh,
                    )
                    for g in range(NCOL):
                        ty = 1 if g == NCOL - 1 else 0
                        nc.tensor.matmul(
                            out=acc[:],
                            lhsT=red_t[
                                :, (row16 * 2 + ty) * 16 : (row16 * 2 + ty + 1) * 16
                            ],
                            rhs=tt[:, g, :],
                            start=(red_i == 0),
                            stop=(red_i == n_red - 1),
                        )
                        red_i += 1

            res_sb = osb.tile([NROW, D], mybir.dt.float32)
            nc.scalar.copy(out=res_sb[:], in_=acc[:])
            nc.sync.dma_start(out=out[:], in_=res_sb[:])

    nc.compile()
    _split_multi_waits(nc)
    return nc


def _host_prep(inputs_pri, inputs_sec, W_pri, W_sec):
    import ml_dtypes

    ip = np.asarray(inputs_pri).astype(np.int64, copy=False)
    is_ = np.asarray(inputs_sec).astype(np.int64, copy=False)
    wp = np.ascontiguousarray(np.asarray(W_pri, dtype=np.float32)).astype(
        ml_dtypes.bfloat16
    )
    ws = np.ascontiguousarray(np.asarray(W_sec, dtype=np.float32)).astype(
        ml_dtypes.bfloat16
    )
    red = _build_red_masks()

    p_ar = np.arange(P)
    in_maps = []
    for k in range(N_CORES):
        idxA = np.zeros((P, N_SLOTS * IDX_COLS), dtype=np.int16)
        for t, idx in enumerate((ip, is_)):
            for r in range(B_LOC):
                row16 = t * B_LOC + r
                seq = idx[k * B_LOC + r]  # [S]
                for s, (c0, nidx, d0, ncol) in enumerate(STREAMS):
                    if c0 >= 0:
                        # slot s -> partition s%128, col c0 + s//128,
                        # position 16*(s%128) + (c0 + s//128)
                        pos = (CPP * p_ar[None, :] + c0 + np.arange(7)[:, None]).reshape(
                            -1
                        )  # [896] in slot order (col-major)
                        stream = (seq[pos] - SPLIT).astype(np.int16)
                        stream = np.concatenate(
                            [stream, np.zeros(nidx - 7 * P, np.int16)]
                        )
                    else:
                        # cols c14, c15, then boundary rights (pos 16p+16,
                        # clamped); final slot (p=127) is masked out of the
                        # reduce -> biased-0 so the trailing-negative trim
                        # never fires.
                        pos = np.concatenate(
                            [
                                CPP * p_ar + 14,
                                CPP * p_ar + 15,
                                np.minimum(CPP * p_ar + CPP, S - 1),
                            ]
                        )
                        stream = (seq[pos] - SPLIT).astype(np.int16)
                        stream[3 * P - 1] = 0
                    slot = row16 * len(STREAMS) + s
                    wrapped = np.tile(stream.reshape(-1, 16).T, (8, 1))
                    idxA[:, slot * IDX_COLS : slot * IDX_COLS + nidx // 16] = wrapped
        in_maps.append(
            {
                "W_pri": wp,
                "W_sec": ws,
                "idxA": idxA,
                "red": red,
            }
        )
    return in_maps


def _run(inputs_pri, inputs_sec, W_pri, W_sec, trace=False):
    global _last_results
    nc = _build_program()
    in_maps = _host_prep(inputs_pri, inputs_sec, W_pri, W_sec)
    res = run_bass_kernel_spmd(nc, in_maps, list(range(N_CORES)), trace=trace)
    _last_results = res
    out = np.empty((2, B, D), dtype=np.float32)
    for k in range(N_CORES):
        o = res.results[k]["out"]  # [16, 256]
        out[0, k * B_LOC : (k + 1) * B_LOC] = o[:B_LOC]
        out[1, k * B_LOC : (k + 1) * B_LOC] = o[B_LOC:]
    return out


def kernel(inputs_pri, inputs_sec, W_pri, W_sec):
    trace = bool(int(os.environ.get("KERNEL_TRACE", "0")))
    return _run(inputs_pri, inputs_sec, W_pri, W_sec, trace=trace)


# revision 8
# speedup vs baseline: 4.3205x; 1.0245x over previous
"""Trainium2 Bass kernel for nn_BiLingual (dual embedding gather + cAddTanh pool).

Computes, for two embedding tables:
    out[t, b, :] = sum_{j=0}^{S-2} tanh(W_t[idx_t[b, j]] + W_t[idx_t[b, j+1]])

Sharding: data-parallel over batch. Each of the 8 cores handles 8 batch rows
for BOTH tables; tables are replicated (converted to bf16 on the host).

Per-core device layout (v3): partition p owns positions [16p, 16p+16) of the
sequence; gathered rows land column-major (slot s -> partition s%128, column
s//128, position = 16*(s%128) + s//128 + 16*c0).  Consecutive positions of a
pair then sit on the SAME partition in ADJACENT columns, so the pairwise add
is a legal free-axis-offset DVE op.  The per-partition boundary pair
(16p+15, 16p+16) gets its right element from one extra 128-idx gather.

Pipeline per sequence row:
  1. dma_gather (gpsimd SWDGE, bf16 tables, 512B/descriptor):
       stream A: cols c=0..3   (512 idxs + 16 biased-0 guards -> junk col 4)
       stream B: cols c=4..7   (512 idxs + 16 guards -> junk col 9)
       stream C: cols c=8..11  (512 idxs + 16 guards -> junk col 14)
       stream D: cols c=12..15 + boundary rights pos 16p+16 (640 idxs ->
                 cols 15-19; final slot is masked and forced to biased-0 so
                 the ucode's trailing-negative trim never fires).
     num_idxs > 1024 hangs the SWDGE ucode (HW-bisected); 4 equal streams
     per sequence keep the 4 SWDGE queues' waves balanced.
     Streams round-robin over the 4 SWDGE queues so all four Q7 core pairs
     generate descriptors concurrently.  int16 index range via biasing:
     base = W[32768:], idx' = idx - 32768.
  2. DVE adds (free-axis column offsets, all partitions base-0):
       seven ops walking the 4-col blocks, skipping the junk cols; the last
       covers pairs c12..c14 plus the boundary pair (valid p<127).
  3. ACT tanh A -> T (bf16), one [128, 4096] instruction per sequence.
  4. PE masked ones-matmul reduces T into a [16, 256] PSUM accumulator
     (output partition = table*8 + local_row); mask ty0 = all partitions
     (in-partition pairs), ty1 = p<127 (boundary column).
"""
import os

import numpy as np

from concourse import bacc, mybir
import concourse.tile as tile
from concourse.bass_utils import run_bass_kernel_spmd

P = 128
B, S, V, D = 64, 2048, 50000, 256
N_CORES = 8
B_LOC = B // N_CORES        # 8 batch rows per core
CPP = 16                    # positions per partition
NCOL = 16                   # result pair-columns per sequence (15 main + 1 boundary)
NROW = 2 * B_LOC            # 16 (table, local row) pairs per core
SPLIT = 32768
N_QUEUES = int(os.environ.get("KQUEUES", "4"))

# per-row gather streams: (first pos-col, n idxs incl guards, dst col, dst ncol)
# num_idxs > 1024 hangs the SWDGE ucode (HW-bisected); 4 equal streams per
# sequence keep all 4 SWDGE queues busy with balanced ~4.8us waves.
STREAMS = [
    (0, 4 * P + 16, 0, 5),      # c0-3  + guards -> junk col 4
    (4, 4 * P + 16, 5, 5),      # c4-7  + guards -> junk col 9
    (8, 4 * P + 16, 10, 5),     # c8-11 + guards -> junk col 14
    (-1, 5 * P, 15, 5),         # c12-15 + boundary rights (no guards)
]
IDX_COLS = 48               # idx tile column pitch per stream (>= 640/16, 32B-aligned)
N_SLOTS = NROW * len(STREAMS)

_last_results = None        # set by _run for test harness introspection


def _build_red_masks():
    # red[:, (row16*2 + ty)*16 : +16]: column row16 holds mask_ty, rest 0.
    # ty=0: all partitions valid (in-partition pairs); ty=1: p < 127 (boundary).
    red = np.zeros((P, NROW * 2 * 16), dtype=np.float32)
    masks = [
        np.ones(P, dtype=np.float32),
        (np.arange(P) < 127).astype(np.float32),
    ]
    for row16 in range(NROW):
        for ty in range(2):
            red[:, (row16 * 2 + ty) * 16 + row16] = masks[ty]
    return red


def _split_multi_waits(nc, max_waits=1):
    """Walrus rejects instructions carrying too many sync waits; hoist excess
    waits onto same-engine NOPs inserted just before the instruction (engine
    program order makes this equivalent)."""
    for bb in nc.main_func.blocks:
        idx = 0
        while idx < len(bb.instructions):
            ins = bb.instructions[idx]
            si = ins.sync_info
            if si is not None and si.on_wait and len(si.on_wait) > max_waits:
                waits = list(si.on_wait)
                extra, keep = waits[:-max_waits], waits[-max_waits:]
                for w0 in range(0, len(extra), max_waits):
                    nop = mybir.InstNoOp(
                        name=nc.get_next_instruction_name(), ins=[], outs=[]
                    )
                    nop.engine = ins.engine
                    nop.sync_info = mybir.SyncInfo(
                        on_wait=extra[w0 : w0 + max_waits], on_update=[]
                    )
                    nc.register_instruction(nop)
                    bb.instructions.insert(idx, nop)
                    idx += 1
                si.on_wait = keep
            idx += 1


def _build_program():
    nc = bacc.Bacc(None, target_bir_lowering=False, num_swdge_queues=N_QUEUES)
    bf16 = mybir.dt.bfloat16
    Wp = nc.declare_dram_parameter("W_pri", [V, D], bf16, isOutput=False)
    Ws = nc.declare_dram_parameter("W_sec", [V, D], bf16, isOutput=False)
    idxA = nc.declare_dram_parameter(
        "idxA", [P, N_SLOTS * IDX_COLS], mybir.dt.int16, isOutput=False
    )
    red = nc.declare_dram_parameter(
        "red", [P, NROW * 2 * 16], mybir.dt.float32, isOutput=False
    )
    out = nc.declare_dram_parameter("out", [NROW, D], mybir.dt.float32, isOutput=True)

    with tile.TileContext(nc) as tc:
        with (
            tc.tile_pool(name="const", bufs=1) as const,
            tc.tile_pool(name="ebuf", bufs=6) as ebuf,
            tc.tile_pool(name="abuf", bufs=4) as abuf,
            tc.tile_pool(name="tbuf", bufs=4) as tbuf,
            tc.tile_pool(name="psR", bufs=1, space="PSUM") as psR,
            tc.tile_pool(name="osb", bufs=1) as osb,
        ):
            red_f32 = const.tile([P, NROW * 2 * 16], mybir.dt.float32)
            nc.sync.dma_start(out=red_f32[:], in_=red[:])
            red_t = const.tile([P, NROW * 2 * 16], bf16)
            nc.vector.tensor_copy(out=red_t[:], in_=red_f32[:])
            iA = const.tile([P, N_SLOTS * IDX_COLS], mybir.dt.int16)
            nc.sync.dma_start(out=iA[:], in_=idxA[:])

            acc = psR.tile([NROW, D], mybir.dt.float32, space="PSUM")
            n_red = NROW * NCOL
            red_i = 0
            q = 0

            for t, W in enumerate((Wp, Ws)):
                for r in range(B_LOC):
                    row16 = t * B_LOC + r
                    e = ebuf.tile([P, 20, D], bf16)
                    for k, (c0, nidx, d0, ncol) in enumerate(STREAMS):
                        slot = row16 * len(STREAMS) + k
                        nc.gpsimd.dma_gather(
                            out_ap=e[:, d0 : d0 + ncol, :],
                            in_ap=W[SPLIT:, :],
                            idxs_ap=iA[
                                :, slot * IDX_COLS : slot * IDX_COLS + nidx // 16
                            ],
                            num_idxs=nidx,
                            num_idxs_reg=nidx,
                            elem_size=D,
                            queue_num=q % N_QUEUES,
                        )
                        q += 1
                    a = abuf.tile([P, NCOL, D], bf16)
                    nc.vector.tensor_add(
                        out=a[:, 0:3, :], in0=e[:, 0:3, :], in1=e[:, 1:4, :]
                    )
                    nc.vector.tensor_add(
                        out=a[:, 3:4, :], in0=e[:, 3:4, :], in1=e[:, 5:6, :]
                    )
                    nc.vector.tensor_add(
                        out=a[:, 4:7, :], in0=e[:, 5:8, :], in1=e[:, 6:9, :]
                    )
                    nc.vector.tensor_add(
                        out=a[:, 7:8, :], in0=e[:, 8:9, :], in1=e[:, 10:11, :]
                    )
                    nc.vector.tensor_add(
                        out=a[:, 8:11, :], in0=e[:, 10:13, :], in1=e[:, 11:14, :]
                    )
                    nc.vector.tensor_add(
                        out=a[:, 11:12, :], in0=e[:, 13:14, :], in1=e[:, 15:16, :]
                    )
                    nc.vector.tensor_add(
                        out=a[:, 12:16, :], in0=e[:, 15:19, :], in1=e[:, 16:20, :]
                    )
                    tt = tbuf.tile([P, NCOL, D], bf16)
                    nc.scalar.activation(
                        tt[:].rearrange("p g d -> p (g d)"),
                        a[:].rearrange("p g d -> p (g d)"),
                        mybir.ActivationFunctionType.Tanh,
                    )
                    for g in range(NCOL):
                        ty = 1 if g == NCOL - 1 else 0
                        nc.tensor.matmul(
                            out=acc[:],
                            lhsT=red_t[
                                :, (row16 * 2 + ty) * 16 : (row16 * 2 + ty + 1) * 16
                            ],
                            rhs=tt[:, g, :],
                            start=(red_i == 0),
                            stop=(red_i == n_red - 1),
                        )
                        red_i += 1

            res_sb = osb.tile([NROW, D], mybir.dt.float32)
            nc.scalar.copy(out=res_sb[:], in_=acc[:])
            nc.sync.dma_start(out=out[:], in_=res_sb[:])

    nc.compile()
    _split_multi_waits(nc)
    return nc


def _host_prep(inputs_pri, inputs_sec, W_pri, W_sec):
    import ml_dtypes

    ip = np.asarray(inputs_pri).astype(np.int64, copy=False)
    is_ = np.asarray(inputs_sec).astype(np.int64, copy=False)
    wp = np.ascontiguousarray(np.asarray(W_pri, dtype=np.float32)).astype(
        ml_dtypes.bfloat16
    )
    ws = np.ascontiguousarray(np.asarray(W_sec, dtype=np.float32)).astype(
        ml_dtypes.bfloat16
    )
    red = _build_red_masks()

    p_ar = np.arange(P)
    in_maps = []
    for k in range(N_CORES):
        idxA = np.zeros((P, N_SLOTS * IDX_COLS), dtype=np.int16)
        for t, idx in enumerate((ip, is_)):
            for r in range(B_LOC):
                row16 = t * B_LOC + r
                seq = idx[k * B_LOC + r]  # [S]
                for s, (c0, nidx, d0, ncol) in enumerate(STREAMS):
                    if c0 >= 0:
                        # slot s -> partition s%128, col c0 + s//128,
                        # position 16*(s%128) + (c0 + s//128)
                        pos = (CPP * p_ar[None, :] + c0 + np.arange(4)[:, None]).reshape(
                            -1
                        )  # [512] in slot order (col-major)
                        stream = (seq[pos] - SPLIT).astype(np.int16)
                        stream = np.concatenate(
                            [stream, np.zeros(nidx - 4 * P, np.int16)]
                        )
                    else:
                        # cols c12-15, then boundary rights (pos 16p+16,
                        # clamped); final slot (p=127) is masked out of the
                        # reduce -> biased-0 so the trailing-negative trim
                        # never fires.
                        pos = np.concatenate(
                            [
                                (CPP * p_ar[None, :] + 12 + np.arange(4)[:, None]).reshape(-1),
                                np.minimum(CPP * p_ar + CPP, S - 1),
                            ]
                        )
                        stream = (seq[pos] - SPLIT).astype(np.int16)
                        stream[5 * P - 1] = 0
                    slot = row16 * len(STREAMS) + s
                    wrapped = np.tile(stream.reshape(-1, 16).T, (8, 1))
                    idxA[:, slot * IDX_COLS : slot * IDX_COLS + nidx // 16] = wrapped
        in_maps.append(
            {
                "W_pri": wp,
                "W_sec": ws,
                "idxA": idxA,
                "red": red,
            }
        )
    return in_maps


def _run(inputs_pri, inputs_sec, W_pri, W_sec, trace=False):
    global _last_results
    nc = _build_program()
    in_maps = _host_prep(inputs_pri, inputs_sec, W_pri, W_sec)
    res = run_bass_kernel_spmd(nc, in_maps, list(range(N_CORES)), trace=trace)
    _last_results = res
    out = np.empty((2, B, D), dtype=np.float32)
    for k in range(N_CORES):
        o = res.results[k]["out"]  # [16, 256]
        out[0, k * B_LOC : (k + 1) * B_LOC] = o[:B_LOC]
        out[1, k * B_LOC : (k + 1) * B_LOC] = o[B_LOC:]
    return out


def kernel(inputs_pri, inputs_sec, W_pri, W_sec):
    trace = bool(int(os.environ.get("KERNEL_TRACE", "0")))
    return _run(inputs_pri, inputs_sec, W_pri, W_sec, trace=trace)
